# revision 20
# baseline (speedup 1.0000x reference)
"""Trainium2 Bass kernel for a dense transformer block (causal attn + MLP).

Problem: B=4, L=2048, D=1024, H=16 (DH=64), DFF=4096, fp32 in/out.

Sharding (no collectives): 8 cores = 4 batches x 2 parity groups.
Core c handles batch b=c//2 and query-row tiles {p, p+2, ..., p+14}
(p=c%2); interleaved 128-row tiles balance causal-attention work between
the two cores of a batch.

v2 structure (single pipelined stream, no DRAM roundtrip):
  A:  LN1 + transposes + K/V full seq, Q own rows (as before)
  B0: attention query-block Bk=0 (ACT-exp bound)
  B1: attention query-block Bk=1, with the entire C0/D0/E0 chain
      (WO proj, LN2, MLP of column block 0) emitted interleaved between
      the per-head blocks so the PE stays saturated while ACT does exp.
  C1/D1/E1: tail for column block 1.
Attention output (avn) stays in SBUF as bf16; the raw own-row residual
x^T arrives host-pre-transposed (x_ownT) straight into the bf16 x2
tiles; WO/W1/W2 all run as bf16 matmuls.
"""

import numpy as np
import ml_dtypes

import concourse.bacc as bacc
import concourse.bass as bass
import concourse.mybir as mybir
import concourse.tile as tile
from concourse.bass_utils import run_bass_kernel_spmd

F32 = mybir.dt.float32
F32R = mybir.dt.float32r
FP8 = mybir.dt.float8e4
BF16 = mybir.dt.bfloat16
BF = ml_dtypes.bfloat16
EPS = 1e-5
AF = mybir.ActivationFunctionType
OP = mybir.AluOpType

B_, L_, D_, H_, DFF_ = 4, 2048, 1024, 16, 4096
N_CORES = 8


def _ja(i, Bk):
    return min(3, max(0, (i - 1 - 8 * Bk) // 2))


def _derived(L, D, H, DFF):
    CT = D // 128
    FT = DFF // 128
    n_lt = L // 128
    n_own = n_lt // 2
    OWN_L = n_own * 128
    assert n_own % 4 == 0
    NB = n_own // 4
    ns = [8 * b + 8 for b in range(NB)]
    moffs = np.cumsum([0] + ns).tolist()
    HT = H // 2
    assert CT == HT
    VW = min(512, D)
    return dict(CT=CT, FT=FT, n_lt=n_lt, n_own=n_own, OWN_L=OWN_L, NB=NB,
                ns=ns, moffs=moffs, NS_TOT=moffs[-1], HT=HT, VW=VW,
                DVB=D // VW, SG=max(1, D // 512))


def build_nc(L=L_, D=D_, H=H_, DFF=DFF_, n_cores=N_CORES):
    g = _derived(L, D, H, DFF)
    CT, FT = g["CT"], g["FT"]
    n_lt, n_own, OWN_L = g["n_lt"], g["n_own"], g["OWN_L"]
    NB, ns, moffs, NS_TOT = g["NB"], g["ns"], g["moffs"], g["NS_TOT"]
    HT, VW, DVB, SG = g["HT"], g["VW"], g["DVB"], g["SG"]
    W = 512
    scale = 1.0 / 8.0  # 1/sqrt(DH)

    nc = bacc.Bacc("TRN2", target_bir_lowering=False, debug=False,
                   num_devices=n_cores)

    dp = nc.declare_dram_parameter
    x_d = dp("x", [L, D], F32, isOutput=False)
    xo_d = dp("x_own", [OWN_L, D], F32, isOutput=False)
    xoT_d = dp("x_ownT", [D, OWN_L], F32, isOutput=False)
    ident_d = dp("ident", [128, 128], F32, isOutput=False)
    wq_d = dp("wq", [128, CT, CT, 128], BF16, isOutput=False)   # [p, d, c, q]
    wk_d = dp("wk", [128, CT, CT, 128], BF16, isOutput=False)
    wv_d = dp("wv", [128, CT, D], BF16, isOutput=False)          # [p, c, dv]
    wo_d = dp("wo", [CT, 128, CT, 128], BF16, isOutput=False)    # [e, p, c, q]
    w1_d = dp("w1", [FT, 128, CT, 128], BF16, isOutput=False)    # [f, p, c, q]
    w2_d = dp("w2", [CT, 128, FT, 128], BF16, isOutput=False)    # [e, p, f, q]
    bq_d = dp("bqc", [128, CT], F32, isOutput=False)
    bk_d = dp("bkc", [128, CT], F32, isOutput=False)
    b1_d = dp("b1c", [128, FT], F32, isOutput=False)
    boeff_d = dp("boeffc", [128, CT], F32, isOutput=False)
    b2_d = dp("b2c", [128, CT], F32, isOutput=False)
    onescv_d = dp("onescv", [128, 1], F32, isOutput=False)
    onesrv_d = dp("onesrv", [1, 128], F32, isOutput=False)
    masks_d = dp("masks", [NS_TOT, 128, 256], BF16, isOutput=False)
    out_d = dp("outT", [D, OWN_L], F32, isOutput=True)

    with tile.TileContext(nc) as tc, \
         nc.allow_low_precision(reason="bf16/f32r matmul operands by design"):
        consts_cm = tc.tile_pool(name="consts", bufs=1)
        consts = consts_cm.__enter__()

        ident_sb = consts.tile([128, 128], F32, tag="ident")
        nc.sync.dma_start(out=ident_sb[:], in_=ident_d[:])
        identb_sb = consts.tile([128, 128], BF16, tag="identb")
        nc.vector.tensor_copy(out=identb_sb[:], in_=ident_sb[:])
        eps_c = consts.tile([128, 1], F32, tag="eps")
        nc.vector.memset(eps_c[:], EPS)
        onesb_c = consts.tile([128, 1], BF16, tag="onesbc")
        nc.vector.memset(onesb_c[:], 1.0)
        ones_c = consts.tile([128, 1], F32R, tag="onesc")
        nc.sync.dma_start(out=ones_c[:], in_=onescv_d[:].bitcast(F32R))
        ones_r = consts.tile([1, 128], F32R, tag="onesr")
        nc.sync.dma_start(out=ones_r[:], in_=onesrv_d[:].bitcast(F32R))
        onesmb = consts.tile([128, 128], BF16, tag="onesmb")
        nc.vector.memset(onesmb[:], 1.0)
        bq_sb = consts.tile([128, CT], F32, tag="bq")
        nc.sync.dma_start(out=bq_sb[:], in_=bq_d[:])
        bk_sb = consts.tile([128, CT], F32, tag="bk")
        nc.sync.dma_start(out=bk_sb[:], in_=bk_d[:])
        b1_sb = consts.tile([128, FT], F32, tag="b1")
        nc.sync.dma_start(out=b1_sb[:], in_=b1_d[:])
        boeff_sb = consts.tile([128, CT], F32, tag="boeff")
        nc.sync.dma_start(out=boeff_sb[:], in_=boeff_d[:])
        b2_sb = consts.tile([128, CT], F32, tag="b2")
        nc.sync.dma_start(out=b2_sb[:], in_=b2_d[:])
        eps_row = consts.tile([1, 1], F32, tag="epsr")
        nc.vector.memset(eps_row[:], EPS)

        attio_cm = tc.tile_pool(name="attio", bufs=1)
        attio = attio_cm.__enter__()
        kt = [attio.tile([128, L], BF16, tag=f"kt{i}", name=f"kt{i}")
              for i in range(CT)]
        qt = [attio.tile([128, OWN_L], BF16, tag=f"qt{i}", name=f"qt{i}")
              for i in range(CT)]
        v_sb = [attio.tile([128, H, 65], BF16, tag=f"v{i}", name=f"v{i}")
                for i in range(n_lt)]

        # ================= Phase A: LN1 + transposes + QKV ===============
        wA_cm = tc.tile_pool(name="workA", bufs=2)
        wA = wA_cm.__enter__()
        quad_cm = tc.tile_pool(name="quad", bufs=1)
        quad = quad_cm.__enter__()
        wqkv_cm = tc.tile_pool(name="wqkv", bufs=1)
        wqkv = wqkv_cm.__enter__()
        psA_t_cm = tc.tile_pool(name="psA_t", bufs=3, space="PSUM")
        psA_t = psA_t_cm.__enter__()
        psA_mm_cm = tc.tile_pool(name="psA_mm", bufs=4, space="PSUM")
        psA_mm = psA_mm_cm.__enter__()

        wk_sb = wqkv.tile([128, CT, CT, 128], BF16, tag="wqk", name="wk_sb")
        nc.sync.dma_start(out=wk_sb[:], in_=wk_d[:])
        wv_sb = wqkv.tile([128, CT, D], BF16, tag="wv", name="wv_sb")
        nc.sync.dma_start(out=wv_sb[:], in_=wv_d[:])

        def ln_transpose_quad(src_d, q):
            """Load 4 seq tiles, plain-LN them (affine folded into weights),
            transpose; returns 8 feature-major [128, 512] bf16 tiles."""
            xns = wA.tile([128, 4, D], F32, tag="xns", bufs=1)
            mvq = wA.tile([128, 4, 2], F32, tag="mvq")
            stdq = wA.tile([128, 4], F32, tag="stdq")
            for t4 in range(4):
                t = 4 * q + t4
                nc.sync.dma_start(out=xns[:, t4, :],
                                  in_=src_d[t * 128:(t + 1) * 128, :])
                stats = wA.tile([128, SG, 6], F32, tag="ln1_stats")
                for s in range(SG):
                    src = (xns[:, t4, s * 512:(s + 1) * 512] if SG > 1
                           else xns[:, t4, :])
                    nc.vector.bn_stats(out=stats[:, s, :], in_=src)
                nc.vector.bn_aggr(out=mvq[:, t4, :], in_=stats[:])
                nc.scalar.activation(out=stdq[:, t4:t4 + 1],
                                     in_=mvq[:, t4, 1:2], func=AF.Sqrt,
                                     bias=eps_c[:])
            rstdq = wA.tile([128, 4], F32, tag="rstdq")
            nc.vector.reciprocal(out=rstdq[:], in_=stdq[:])
            for t4 in range(4):
                nc.vector.tensor_scalar(out=xns[:, t4, :], in0=xns[:, t4, :],
                                        scalar1=mvq[:, t4, 0:1],
                                        scalar2=rstdq[:, t4:t4 + 1],
                                        op0=OP.subtract, op1=OP.mult)
            xq = [quad.tile([128, 512], BF16, tag=f"xq{ci}", name=f"xq{ci}")
                  for ci in range(CT)]
            for ci in range(CT):
                psT4 = psA_t.tile([128, 512], F32, tag="ps_t")
                for t4 in range(4):
                    nc.tensor.matmul(psT4[:, t4 * 128:(t4 + 1) * 128],
                                     xns[:, t4, ci * 128:(ci + 1) * 128],
                                     ident_sb[:], is_transpose=True,
                                     start=(t4 == 0), stop=(t4 == 3))
                nc.vector.tensor_copy(out=xq[ci][:], in_=psT4[:])
            return xq

        # full sequence: KT + V
        for q in range(n_lt // 4):
            xq = ln_transpose_quad(x_d, q)
            for di in range(CT):
                ps = psA_mm.tile([128, W], F32, tag="ps_mm")
                for ci in range(CT):
                    nc.tensor.matmul(ps[:], wk_sb[:, di, ci, :], xq[ci][:],
                                     start=(ci == 0), stop=(ci == CT - 1))
                nc.vector.tensor_scalar_add(
                    out=kt[di][:, q * 512:(q + 1) * 512], in0=ps[:],
                    scalar1=bk_sb[:, di:di + 1])
            for st4 in range(4):
                st = 4 * q + st4
                nc.vector.memset(v_sb[st][:, :, 64:65], 1.0)
                for vb in range(DVB):
                    ps = psA_mm.tile([128, VW], F32, tag="ps_mm")
                    for ci in range(CT):
                        nc.tensor.matmul(
                            ps[:], xq[ci][:, st4 * 128:(st4 + 1) * 128],
                            wv_sb[:, ci, vb * VW:(vb + 1) * VW],
                            start=(ci == 0), stop=(ci == CT - 1))
                    nhh = VW // 64
                    nc.vector.tensor_copy(
                        out=v_sb[st][:, vb * nhh:(vb + 1) * nhh, 0:64],
                        in_=ps[:].rearrange("p (h d) -> p h d", d=64))
            if q == n_lt // 4 - 1:
                # K weights done; reuse the slot for Q weights
                wq_sb = wqkv.tile([128, CT, CT, 128], BF16, tag="wqk",
                                  name="wq_sb")
                nc.sync.dma_start(out=wq_sb[:], in_=wq_d[:])

        # own rows: QT
        for q in range(n_own // 4):
            xqo = ln_transpose_quad(xo_d, q)
            for di in range(CT):
                ps = psA_mm.tile([128, W], F32, tag="ps_mm")
                for ci in range(CT):
                    nc.tensor.matmul(ps[:], wq_sb[:, di, ci, :], xqo[ci][:],
                                     start=(ci == 0), stop=(ci == CT - 1))
                nc.vector.tensor_scalar_add(
                    out=qt[di][:, q * 512:(q + 1) * 512], in0=ps[:],
                    scalar1=bq_sb[:, di:di + 1])

        for cm in (psA_mm_cm, psA_t_cm, wqkv_cm, quad_cm, wA_cm):
            cm.__exit__(None, None, None)

        # =========== Long-lived pools for B/C/D/E ========================
        maskp_cm = tc.tile_pool(name="maskp", bufs=1)
        maskp = maskp_cm.__enter__()
        avn_cm = tc.tile_pool(name="avnp", bufs=2)
        avnp = avn_cm.__enter__()
        x2_cm = tc.tile_pool(name="x2p", bufs=1)
        x2p = x2_cm.__enter__()
        x2n_cm = tc.tile_pool(name="x2np", bufs=1)
        x2np = x2n_cm.__enter__()
        h_cm = tc.tile_pool(name="hp", bufs=1)
        hpool = h_cm.__enter__()
        wB_cm = tc.tile_pool(name="workB", bufs=2)
        wB = wB_cm.__enter__()
        wR_cm = tc.tile_pool(name="rec", bufs=2)
        wR = wR_cm.__enter__()
        wk2_cm = tc.tile_pool(name="work2", bufs=2)
        wk2 = wk2_cm.__enter__()
        psB_sc_cm = tc.tile_pool(name="psB_sc", bufs=2, space="PSUM")
        psB_sc = psB_sc_cm.__enter__()

        MNS = max(ns)

        def load_masks(Bk):
            n_s, mo = ns[Bk], moffs[Bk]
            mk = maskp.tile([128, MNS, 256], BF16, tag="masks")
            nc.sync.dma_start(
                out=mk[:, 0:n_s, :],
                in_=masks_d[mo:mo + n_s].rearrange("t p f -> p t f"))
            return mk

        def attn_block(Bk, ht, av_pool, mk, avn_t, pend, tails):
            """One (query-block, head-pair) attention block. Emits the
            deferred norm_tail `pend` at i==2 if given."""
            n_s = ns[Bk]
            ps_av = av_pool.tile([128, 2 * W], F32, tag="ps_av",
                                 name="ps_av")

            def av_pair(pi, pex, plo):
                for hp in range(2):
                    nc.tensor.matmul(ps_av[0:65, hp * W + plo:(hp + 1) * W],
                                     v_sb[pi][:, 2 * ht + hp, :],
                                     pex[:, hp * W + plo:(hp + 1) * W],
                                     start=(pi == 0),
                                     stop=(pi == n_s - 1))

            prev = None
            for i in range(n_s):
                lo = _ja(i, Bk) * 128
                ps_sc = psB_sc.tile([128, 2 * W], F32, tag="ps_sc",
                                    name="ps_sc")
                for hp in range(2):
                    nc.tensor.matmul(
                        ps_sc[:, hp * W + lo:(hp + 1) * W],
                        kt[ht][64 * hp:64 * hp + 64,
                               i * 128:(i + 1) * 128],
                        qt[ht][64 * hp:64 * hp + 64,
                               Bk * W + lo:(Bk + 1) * W],
                        start=True, stop=True)
                ex = wB.tile([128, 2 * W], BF16, tag="exp", name="ex")
                nc.scalar.activation(
                    out=ex[:].rearrange("p (h w) -> p h w", h=2)[:, :, lo:W],
                    in_=ps_sc[:].rearrange("p (h w) -> p h w", h=2)[:, :, lo:W],
                    func=AF.Exp, scale=scale)
                mw = min(W - lo, 256)
                for hp in range(2):
                    nc.vector.tensor_mul(
                        ex[:, hp * W + lo:hp * W + lo + mw],
                        ex[:, hp * W + lo:hp * W + lo + mw],
                        mk[:, i, 0:mw])
                if prev is not None:
                    av_pair(*prev)
                prev = (i, ex, lo)
                if i == 2 and pend is not None:
                    tails(pend)
                    pend = None
            av_pair(*prev)
            if pend is not None:
                tails(pend)
            # 1/sumexp straight from the PSUM rows (exact recip, bf16 out)
            rec = wR.tile([128, 2 * W], BF16, tag="rec")
            for hp in range(2):
                nc.vector.reciprocal(
                    out=rec[64:65, hp * W:(hp + 1) * W],
                    in_=ps_av[64:65, hp * W:(hp + 1) * W])
            return (ht, Bk, ps_av, rec, avn_t)

        def norm_tail(st):
            t_ht, t_Bk, t_av, t_rec, t_avn = st
            for hp in range(2):
                ps_bc = psB_sc.tile([128, 2 * W], F32, tag="ps_sc",
                                    name="bc")
                nc.tensor.matmul(ps_bc[0:64, 0:W], onesmb[64:65, 0:64],
                                 t_rec[64:65, hp * W:(hp + 1) * W],
                                 start=True, stop=True)
                bc_sb = wR.tile([64, W], F32, tag=f"bc_sb{hp}", bufs=1,
                                name=f"bc_sb{hp}")
                nc.vector.tensor_copy(out=bc_sb[:], in_=ps_bc[0:64, 0:W])
                nc.vector.tensor_mul(
                    t_avn[t_ht][64 * hp:64 * hp + 64, :],
                    t_av[0:64, hp * W:(hp + 1) * W], bc_sb[:])

        # =================== B0: attention block Bk=0 ====================
        psB_av0_cm = tc.tile_pool(name="psB_av0", bufs=2, space="PSUM")
        psB_av0 = psB_av0_cm.__enter__()

        mk0 = load_masks(0)
        avn0 = [avnp.tile([128, W], BF16, tag=f"avn{ci}", name=f"avn{ci}")
                for ci in range(CT)]
        # raw residual x^T (column block 0) straight from DRAM
        x2_0 = [x2p.tile([128, W], F32R, tag=f"x2_{ci}", name=f"x2_{ci}")
                for ci in range(CT)]
        for ci in range(CT):
            nc.sync.dma_start(
                out=x2_0[ci][:],
                in_=xoT_d[ci * 128:(ci + 1) * 128, 0:W].bitcast(F32R))

        pending = None
        for ht in range(HT):
            pending = attn_block(0, ht, psB_av0, mk0, avn0, pending,
                                 norm_tail)
        norm_tail(pending)

        psB_av0_cm.__exit__(None, None, None)

        # ============ B1 (Bk=1) interleaved with C0/D0/E0 ================
        psCDE_cm = tc.tile_pool(name="psCDE", bufs=2, space="PSUM")
        psCDE = psCDE_cm.__enter__()
        psB_av1_cm = tc.tile_pool(name="psB_av1", bufs=1, space="PSUM")
        psB_av1 = psB_av1_cm.__enter__()

        def wo_proj(nb, x2_nb, avn_nb, eis):
            for ei in eis:
                wt = wk2.tile([128, CT, 128], BF16, tag="wot")
                nc.sync.dma_start(out=wt[:], in_=wo_d[ei])
                ps = psCDE.tile([128, W], F32, tag="acc", name="ps_o")
                for ci in range(CT):
                    nc.tensor.matmul(ps[:], wt[:, ci, :],
                                     avn_nb[ci][:],
                                     start=(ci == 0), stop=(ci == CT - 1))
                nc.vector.scalar_tensor_tensor(
                    out=x2_nb[ei][:], in0=ps[:],
                    scalar=boeff_sb[:, ei:ei + 1],
                    in1=x2_nb[ei][:], op0=OP.add, op1=OP.add)

        def ln2(nb, x2_nb, x2n_nb):
            ps_mu = psCDE.tile([128, W], F32, tag="acc", name="ps_mu")
            ps_sq = psCDE.tile([128, W], F32, tag="acc", name="ps_sq")
            for ci in range(CT):
                nc.tensor.matmul(ps_mu[0:1, :], ones_c[:], x2_nb[ci][:],
                                 start=(ci == 0), stop=(ci == CT - 1))
                sq = wk2.tile([128, W], F32R, tag="sq", bufs=1)
                nc.scalar.activation(out=sq[:], in_=x2_nb[ci][:],
                                     func=AF.Square)
                nc.tensor.matmul(ps_sq[0:1, :], ones_c[:], sq[:],
                                 start=(ci == 0), stop=(ci == CT - 1))
            mur = wk2.tile([1, W], F32R, tag="mur", bufs=1)
            nc.vector.tensor_scalar_mul(out=mur[:], in0=ps_mu[0:1, :],
                                        scalar1=1.0 / D)
            mu2 = wk2.tile([1, W], F32, tag="mu2", bufs=1)
            nc.vector.tensor_mul(mu2[:], mur[:], mur[:])
            varr = wk2.tile([1, W], F32, tag="varr", bufs=1)
            nc.vector.tensor_scalar_mul(out=varr[:], in0=ps_sq[0:1, :],
                                        scalar1=1.0 / D)
            nc.vector.tensor_sub(varr[:], varr[:], mu2[:])
            stdr = wk2.tile([1, W], F32, tag="stdr", bufs=1)
            nc.scalar.activation(out=stdr[:], in_=varr[:], func=AF.Sqrt,
                                 bias=eps_row[:])
            rstdr = wk2.tile([1, W], F32R, tag="rstdr", bufs=1)
            nc.vector.reciprocal(out=rstdr[:], in_=stdr[:])
            ps_mub = psCDE.tile([128, W], F32, tag="acc", name="ps_mub")
            nc.tensor.matmul(ps_mub[:], ones_r[:], mur[:],
                             start=True, stop=True)
            ps_rsb = psCDE.tile([128, W], F32, tag="acc", name="ps_rsb")
            nc.tensor.matmul(ps_rsb[:], ones_r[:], rstdr[:],
                             start=True, stop=True)
            for ci in range(CT):
                t1 = wk2.tile([128, W], F32, tag="t1", bufs=1)
                nc.vector.tensor_sub(t1[:], x2_nb[ci][:], ps_mub[:])
                nc.vector.tensor_mul(x2n_nb[ci][:], t1[:], ps_rsb[:])

        def mlp_w1(nb, x2n_nb, h_nb, fs):
            for f in fs:
                wtile = wk2.tile([128, CT, 128], BF16, tag="w1t")
                nc.sync.dma_start(out=wtile[:], in_=w1_d[f])
                ps = psCDE.tile([128, W], F32, tag="acc", name="ps_h")
                for ci in range(CT):
                    nc.tensor.matmul(ps[:], wtile[:, ci, :], x2n_nb[ci][:],
                                     start=(ci == 0), stop=(ci == CT - 1))
                nc.scalar.activation(out=h_nb[f][:], in_=ps[:],
                                     func=AF.Relu, bias=b1_sb[:, f:f + 1])

        def mlp_w2(nb, h_nb, x2_nb, eis):
            FH = FT // 4
            for ei in eis:
                ps = psCDE.tile([128, W], F32, tag="acc", name="ps_o2")
                for half in range(4):
                    wtile = wk2.tile([128, FH, 128], BF16, tag="w2t")
                    nc.sync.dma_start(
                        out=wtile[:],
                        in_=w2_d[ei][:, half * FH:(half + 1) * FH, :])
                    for fh in range(FH):
                        f = half * FH + fh
                        nc.tensor.matmul(ps[:], wtile[:, fh, :],
                                         h_nb[f][:],
                                         start=(f == 0), stop=(f == FT - 1))
                osb = wk2.tile([128, W], F32, tag="osb", bufs=1)
                nc.vector.scalar_tensor_tensor(
                    out=osb[:], in0=ps[:], scalar=b2_sb[:, ei:ei + 1],
                    in1=x2_nb[ei][:], op0=OP.add, op1=OP.add)
                nc.sync.dma_start(
                    out=out_d[ei * 128:(ei + 1) * 128, nb * W:(nb + 1) * W],
                    in_=osb[:])

        mk1 = load_masks(1)
        avn1 = [avnp.tile([128, W], BF16, tag=f"avn{ci}", name=f"avn{ci}b")
                for ci in range(CT)]
        x2n_0 = [x2np.tile([128, W], BF16, tag=f"x2n{ci}",
                           name=f"x2n{ci}") for ci in range(CT)]
        h_0 = [hpool.tile([128, W], BF16, tag=f"h{f}", name=f"h{f}")
               for f in range(FT)]

        chunks = [
            lambda: wo_proj(0, x2_0, avn0, range(CT)),
            lambda: ln2(0, x2_0, x2n_0),
            lambda: mlp_w1(0, x2n_0, h_0, range(0, 8)),
            lambda: mlp_w1(0, x2n_0, h_0, range(8, 16)),
            lambda: mlp_w1(0, x2n_0, h_0, range(16, 24)),
            lambda: mlp_w1(0, x2n_0, h_0, range(24, 32)),
            lambda: mlp_w2(0, h_0, x2_0, range(0, 4)),
            lambda: mlp_w2(0, h_0, x2_0, range(4, 8)),
        ]
        for ht in range(HT):
            st = attn_block(1, ht, psB_av1, mk1, avn1, None, norm_tail)
            chunks[ht]()
            norm_tail(st)

        # ===================== tail: C1 / D1 / E1 ========================
        x2_1 = [x2p.tile([128, W], F32R, tag=f"x2_{ci}", name=f"x2_{ci}b")
                for ci in range(CT)]
        for ci in range(CT):
            nc.sync.dma_start(
                out=x2_1[ci][:],
                in_=xoT_d[ci * 128:(ci + 1) * 128, W:2 * W].bitcast(F32R))
        wo_proj(1, x2_1, avn1, range(CT))
        x2n_1 = [x2np.tile([128, W], BF16, tag=f"x2n{ci}",
                           name=f"x2n{ci}b") for ci in range(CT)]
        ln2(1, x2_1, x2n_1)
        h_1 = [hpool.tile([128, W], BF16, tag=f"h{f}", name=f"h{f}b")
               for f in range(FT)]
        mlp_w1(1, x2n_1, h_1, range(FT))
        mlp_w2(1, h_1, x2_1, range(CT))

        for cm in (psB_av1_cm, psCDE_cm, psB_sc_cm, wk2_cm, wR_cm, wB_cm,
                   h_cm, x2n_cm, x2_cm, avn_cm, maskp_cm, attio_cm,
                   consts_cm):
            cm.__exit__(None, None, None)

    nc.compile()
    return nc, g


def make_masks(p, n_own):
    """Per-parity boundary masks [NS_TOT, 128, 512] covering the two
    chunks starting at ja(i, Bk) (bf16 0/1)."""
    NB = n_own // 4
    out = []
    for Bk in range(NB):
        n_s = 8 * Bk + 8
        m = np.ones((n_s, 128, 256), np.float32)
        for i in range(n_s):
            ja = _ja(i, Bk)
            sg = 128 * i + np.arange(128)[:, None]
            for k in range(2):
                j = ja + k
                if j > 3:
                    continue
                gidx = p + 2 * (4 * Bk + j)
                lg = 128 * gidx + np.arange(128)[None, :]
                m[i][:, 128 * k:128 * (k + 1)] = (sg <= lg)
        out.append(m)
    return np.concatenate(out, 0).astype(BF)


def _tile_lhsT(wmat):
    """[K, M] -> [m, p, c, q] with out[m, p, c, q] = wmat[128c+p, 128m+q]."""
    K, M = wmat.shape
    CT, MT = K // 128, M // 128
    w = wmat.reshape(CT, 128, MT, 128)
    return np.ascontiguousarray(w.transpose(2, 1, 0, 3))


def prep_in_maps(inputs, L=L_, D=D_, H=H_, DFF=DFF_, Bn=B_):
    f64 = lambda k: np.asarray(inputs[k], np.float64)
    X = np.asarray(inputs["X"], np.float32)
    WQ, WK, WV, WO = f64("WQ"), f64("WK"), f64("WV"), f64("WO")
    W1, W2 = f64("W1"), f64("W2")
    bQ, bK, bV, bO = f64("bQ"), f64("bK"), f64("bV"), f64("bO")
    b1, b2 = f64("b1"), f64("b2")
    g1, be1, g2, be2 = f64("g1"), f64("be1"), f64("g2"), f64("be2")

    g = _derived(L, D, H, DFF)
    CT, FT, n_own = g["CT"], g["FT"], g["n_own"]

    # fold LayerNorm affine transforms into the downstream weights
    WQf, bQf = g1[:, None] * WQ, bQ + be1 @ WQ
    WKf, bKf = g1[:, None] * WK, bK + be1 @ WK
    WVf, bVf = g1[:, None] * WV, bV + be1 @ WV
    boeff = bO + WO.T @ bVf
    # scale h by 4 (into fp8's normal range); fold 1/4 into W2
    HS = 4.0
    W1f, b1f = g2[:, None] * W1 * HS, (b1 + be2 @ W1) * HS
    W2 = W2 / HS

    c32 = lambda a: np.ascontiguousarray(a).astype(np.float32)
    wq_t = np.ascontiguousarray(
        _tile_lhsT(WQf).transpose(1, 0, 2, 3)).astype(BF)
    wk_t = np.ascontiguousarray(
        _tile_lhsT(WKf).transpose(1, 0, 2, 3)).astype(BF)
    wv_r = np.ascontiguousarray(
        WVf.reshape(CT, 128, D).transpose(1, 0, 2)).astype(BF)
    wo_t = _tile_lhsT(WO).astype(BF)
    w1_t = _tile_lhsT(W1f).astype(BF)
    w2_t = _tile_lhsT(W2).astype(BF)

    def cols(v, nt):
        return c32(np.reshape(v, (nt, 128)).T)

    common = dict(
        ident=np.eye(128, dtype=np.float32),
        wq=wq_t, wk=wk_t, wv=wv_r, wo=wo_t, w1=w1_t, w2=w2_t,
        bqc=cols(bQf, CT), bkc=cols(bKf, CT), b1c=cols(b1f, FT),
        boeffc=cols(boeff, CT), b2c=cols(b2, CT),
        onescv=np.ones((128, 1), np.float32),
        onesrv=np.ones((1, 128), np.float32),
    )
    masks_by_p = [make_masks(p, n_own) for p in range(2)]

    in_maps = []
    for core in range(2 * Bn):
        b, p = core // 2, core % 2
        own_rows = np.concatenate(
            [np.arange(128 * (p + 2 * k), 128 * (p + 2 * k) + 128)
             for k in range(n_own)])
        m = dict(common)
        m["x"] = np.ascontiguousarray(X[b])
        m["x_own"] = np.ascontiguousarray(X[b][own_rows])
        m["x_ownT"] = np.ascontiguousarray(X[b][own_rows].T)
        m["masks"] = masks_by_p[p]
        in_maps.append(m)
    return in_maps


def gather(results, L=L_, D=D_, Bn=B_):
    n_own = (L // 128) // 2
    out = np.empty((Bn, L, D), np.float32)
    for core, r in enumerate(results):
        b, p = core // 2, core % 2
        part = np.ascontiguousarray(r["outT"].T)
        for k in range(n_own):
            out[b, 128 * (p + 2 * k):128 * (p + 2 * k) + 128, :] = \
                part[128 * k:128 * (k + 1), :]
    return out


_NC_CACHE = {}


def get_nc():
    if "nc" not in _NC_CACHE:
        _NC_CACHE["nc"] = build_nc()
    return _NC_CACHE["nc"]


def kernel(**inputs) -> np.ndarray:
    nc, _ = get_nc()
    in_maps = prep_in_maps(inputs)
    res = run_bass_kernel_spmd(nc, in_maps, list(range(N_CORES)))
    return gather(res.results)


# revision 21
# speedup vs baseline: 1.1241x; 1.1241x over previous
"""Trainium2 Bass kernel for a dense transformer block (causal attn + MLP).

Problem: B=4, L=2048, D=1024, H=16 (DH=64), DFF=4096, fp32 in/out.

Sharding (no collectives): 8 cores = 4 batches x 2 parity groups.
Core c handles batch b=c//2 and query-row tiles {p, p+2, ..., p+14}
(p=c%2); interleaved 128-row tiles balance causal-attention work between
the two cores of a batch.

v2 structure (single pipelined stream, no DRAM roundtrip):
  A:  LN1 + transposes + K/V full seq, Q own rows (as before)
  B0: attention query-block Bk=0 (ACT-exp bound)
  B1: attention query-block Bk=1, with the entire C0/D0/E0 chain
      (WO proj, LN2, MLP of column block 0) emitted interleaved between
      the per-head blocks so the PE stays saturated while ACT does exp.
  C1/D1/E1: tail for column block 1.
Attention output (avn) stays in SBUF as bf16; the raw own-row residual
x^T arrives host-pre-transposed (x_ownT) straight into the bf16 x2
tiles; WO/W1/W2 all run as bf16 matmuls.
"""

import numpy as np
import ml_dtypes

import concourse.bacc as bacc
import concourse.bass as bass
import concourse.mybir as mybir
import concourse.tile as tile
from concourse.bass_utils import run_bass_kernel_spmd

F32 = mybir.dt.float32
F32R = mybir.dt.float32r
FP8 = mybir.dt.float8e4
BF16 = mybir.dt.bfloat16
BF = ml_dtypes.bfloat16
EPS = 1e-5
AF = mybir.ActivationFunctionType
OP = mybir.AluOpType

B_, L_, D_, H_, DFF_ = 4, 2048, 1024, 16, 4096
N_CORES = 8


def _ja(i, Bk):
    return min(3, max(0, (i - 1 - 8 * Bk) // 2))


def _derived(L, D, H, DFF):
    CT = D // 128
    FT = DFF // 128
    n_lt = L // 128
    n_own = n_lt // 2
    OWN_L = n_own * 128
    assert n_own % 4 == 0
    NB = n_own // 4
    ns = [8 * b + 8 for b in range(NB)]
    moffs = np.cumsum([0] + ns).tolist()
    HT = H // 2
    assert CT == HT
    VW = min(512, D)
    return dict(CT=CT, FT=FT, n_lt=n_lt, n_own=n_own, OWN_L=OWN_L, NB=NB,
                ns=ns, moffs=moffs, NS_TOT=moffs[-1], HT=HT, VW=VW,
                DVB=D // VW, SG=max(1, D // 512))


def build_nc(L=L_, D=D_, H=H_, DFF=DFF_, n_cores=N_CORES):
    g = _derived(L, D, H, DFF)
    CT, FT = g["CT"], g["FT"]
    n_lt, n_own, OWN_L = g["n_lt"], g["n_own"], g["OWN_L"]
    NB, ns, moffs, NS_TOT = g["NB"], g["ns"], g["moffs"], g["NS_TOT"]
    HT, VW, DVB, SG = g["HT"], g["VW"], g["DVB"], g["SG"]
    W = 512
    scale = 1.0 / 8.0  # 1/sqrt(DH)

    nc = bacc.Bacc("TRN2", target_bir_lowering=False, debug=False,
                   num_devices=n_cores)

    dp = nc.declare_dram_parameter
    x_d = dp("x", [L, D], F32, isOutput=False)
    xo_d = dp("x_own", [OWN_L, D], F32, isOutput=False)
    xoT_d = dp("x_ownT", [D, OWN_L], F32, isOutput=False)
    ident_d = dp("ident", [128, 128], F32, isOutput=False)
    wq_d = dp("wq", [128, CT, CT, 128], BF16, isOutput=False)   # [p, d, c, q]
    wk_d = dp("wk", [128, CT, CT, 128], BF16, isOutput=False)
    wv_d = dp("wv", [128, CT, D], BF16, isOutput=False)          # [p, c, dv]
    wo_d = dp("wo", [CT, 128, CT, 128], BF16, isOutput=False)    # [e, p, c, q]
    w1_d = dp("w1", [FT, 128, CT, 128], BF16, isOutput=False)    # [f, p, c, q]
    w2_d = dp("w2", [CT, 128, FT, 128], BF16, isOutput=False)    # [e, p, f, q]
    bq_d = dp("bqc", [128, CT], F32, isOutput=False)
    bk_d = dp("bkc", [128, CT], F32, isOutput=False)
    b1_d = dp("b1c", [128, FT], F32, isOutput=False)
    boeff_d = dp("boeffc", [128, CT], F32, isOutput=False)
    b2_d = dp("b2c", [128, CT], F32, isOutput=False)
    onescv_d = dp("onescv", [128, 1], F32, isOutput=False)
    onesrv_d = dp("onesrv", [1, 128], F32, isOutput=False)
    masks_d = dp("masks", [NS_TOT, 128, 256], BF16, isOutput=False)
    out_d = dp("outT", [D, OWN_L], F32, isOutput=True)

    with tile.TileContext(nc) as tc, \
         nc.allow_low_precision(reason="bf16/f32r matmul operands by design"):
        consts_cm = tc.tile_pool(name="consts", bufs=1)
        consts = consts_cm.__enter__()

        ident_sb = consts.tile([128, 128], F32, tag="ident")
        nc.sync.dma_start(out=ident_sb[:], in_=ident_d[:])
        identb_sb = consts.tile([128, 128], BF16, tag="identb")
        nc.vector.tensor_copy(out=identb_sb[:], in_=ident_sb[:])
        eps_c = consts.tile([128, 1], F32, tag="eps")
        nc.vector.memset(eps_c[:], EPS)
        onesb_c = consts.tile([128, 1], BF16, tag="onesbc")
        nc.vector.memset(onesb_c[:], 1.0)
        ones_c = consts.tile([128, 1], F32R, tag="onesc")
        nc.sync.dma_start(out=ones_c[:], in_=onescv_d[:].bitcast(F32R))
        ones_r = consts.tile([1, 128], F32R, tag="onesr")
        nc.sync.dma_start(out=ones_r[:], in_=onesrv_d[:].bitcast(F32R))
        onesmb = consts.tile([128, 128], BF16, tag="onesmb")
        nc.vector.memset(onesmb[:], 1.0)
        bq_sb = consts.tile([128, CT], F32, tag="bq")
        nc.sync.dma_start(out=bq_sb[:], in_=bq_d[:])
        bk_sb = consts.tile([128, CT], F32, tag="bk")
        nc.sync.dma_start(out=bk_sb[:], in_=bk_d[:])
        b1_sb = consts.tile([128, FT], F32, tag="b1")
        nc.sync.dma_start(out=b1_sb[:], in_=b1_d[:])
        boeff_sb = consts.tile([128, CT], F32, tag="boeff")
        nc.sync.dma_start(out=boeff_sb[:], in_=boeff_d[:])
        b2_sb = consts.tile([128, CT], F32, tag="b2")
        nc.sync.dma_start(out=b2_sb[:], in_=b2_d[:])
        eps_row = consts.tile([1, 1], F32, tag="epsr")
        nc.vector.memset(eps_row[:], EPS)

        attio_cm = tc.tile_pool(name="attio", bufs=1)
        attio = attio_cm.__enter__()
        kt = [attio.tile([128, L], BF16, tag=f"kt{i}", name=f"kt{i}")
              for i in range(CT)]
        qt = [attio.tile([128, OWN_L], BF16, tag=f"qt{i}", name=f"qt{i}")
              for i in range(CT)]
        v_sb = [attio.tile([128, H, 65], BF16, tag=f"v{i}", name=f"v{i}")
                for i in range(n_lt)]

        # ================= Phase A: LN1 + transposes + QKV ===============
        wA_cm = tc.tile_pool(name="workA", bufs=2)
        wA = wA_cm.__enter__()
        quad_cm = tc.tile_pool(name="quad", bufs=1)
        quad = quad_cm.__enter__()
        wqkv_cm = tc.tile_pool(name="wqkv", bufs=1)
        wqkv = wqkv_cm.__enter__()
        psA_t_cm = tc.tile_pool(name="psA_t", bufs=3, space="PSUM")
        psA_t = psA_t_cm.__enter__()
        psA_mm_cm = tc.tile_pool(name="psA_mm", bufs=4, space="PSUM")
        psA_mm = psA_mm_cm.__enter__()

        wk_sb = wqkv.tile([128, CT, CT, 128], BF16, tag="wqk", name="wk_sb")
        nc.sync.dma_start(out=wk_sb[:], in_=wk_d[:])
        wv_sb = wqkv.tile([128, CT, D], BF16, tag="wv", name="wv_sb")
        nc.sync.dma_start(out=wv_sb[:], in_=wv_d[:])

        def ln_transpose_quad(src_d, q):
            """Load 4 seq tiles, plain-LN them (affine folded into weights),
            transpose; returns 8 feature-major [128, 512] bf16 tiles."""
            xns = wA.tile([128, 4, D], F32, tag="xns", bufs=1)
            mvq = wA.tile([128, 4, 2], F32, tag="mvq")
            stdq = wA.tile([128, 4], F32, tag="stdq")
            for t4 in range(4):
                t = 4 * q + t4
                nc.sync.dma_start(out=xns[:, t4, :],
                                  in_=src_d[t * 128:(t + 1) * 128, :])
                stats = wA.tile([128, SG, 6], F32, tag="ln1_stats")
                for s in range(SG):
                    src = (xns[:, t4, s * 512:(s + 1) * 512] if SG > 1
                           else xns[:, t4, :])
                    nc.vector.bn_stats(out=stats[:, s, :], in_=src)
                nc.vector.bn_aggr(out=mvq[:, t4, :], in_=stats[:])
                nc.scalar.activation(out=stdq[:, t4:t4 + 1],
                                     in_=mvq[:, t4, 1:2], func=AF.Sqrt,
                                     bias=eps_c[:])
            rstdq = wA.tile([128, 4], F32, tag="rstdq")
            nc.vector.reciprocal(out=rstdq[:], in_=stdq[:])
            for t4 in range(4):
                nc.vector.tensor_scalar(out=xns[:, t4, :], in0=xns[:, t4, :],
                                        scalar1=mvq[:, t4, 0:1],
                                        scalar2=rstdq[:, t4:t4 + 1],
                                        op0=OP.subtract, op1=OP.mult)
            xq = [quad.tile([128, 512], BF16, tag=f"xq{ci}", name=f"xq{ci}")
                  for ci in range(CT)]
            for ci in range(CT):
                psT4 = psA_t.tile([128, 512], F32, tag="ps_t")
                for t4 in range(4):
                    nc.tensor.matmul(psT4[:, t4 * 128:(t4 + 1) * 128],
                                     xns[:, t4, ci * 128:(ci + 1) * 128],
                                     ident_sb[:], is_transpose=True,
                                     start=(t4 == 0), stop=(t4 == 3))
                nc.vector.tensor_copy(out=xq[ci][:], in_=psT4[:])
            return xq

        # full sequence: KT + V
        for q in range(n_lt // 4):
            xq = ln_transpose_quad(x_d, q)
            for di in range(CT):
                ps = psA_mm.tile([128, W], F32, tag="ps_mm")
                for ci in range(CT):
                    nc.tensor.matmul(ps[:], wk_sb[:, di, ci, :], xq[ci][:],
                                     start=(ci == 0), stop=(ci == CT - 1))
                nc.vector.tensor_scalar_add(
                    out=kt[di][:, q * 512:(q + 1) * 512], in0=ps[:],
                    scalar1=bk_sb[:, di:di + 1])
            for st4 in range(4):
                st = 4 * q + st4
                nc.vector.memset(v_sb[st][:, :, 64:65], 1.0)
                for vb in range(DVB):
                    ps = psA_mm.tile([128, VW], F32, tag="ps_mm")
                    for ci in range(CT):
                        nc.tensor.matmul(
                            ps[:], xq[ci][:, st4 * 128:(st4 + 1) * 128],
                            wv_sb[:, ci, vb * VW:(vb + 1) * VW],
                            start=(ci == 0), stop=(ci == CT - 1))
                    nhh = VW // 64
                    nc.vector.tensor_copy(
                        out=v_sb[st][:, vb * nhh:(vb + 1) * nhh, 0:64],
                        in_=ps[:].rearrange("p (h d) -> p h d", d=64))
            if q == n_lt // 4 - 1:
                # K weights done; reuse the slot for Q weights
                wq_sb = wqkv.tile([128, CT, CT, 128], BF16, tag="wqk",
                                  name="wq_sb")
                nc.sync.dma_start(out=wq_sb[:], in_=wq_d[:])

        # own rows: QT
        for q in range(n_own // 4):
            xqo = ln_transpose_quad(xo_d, q)
            for di in range(CT):
                ps = psA_mm.tile([128, W], F32, tag="ps_mm")
                for ci in range(CT):
                    nc.tensor.matmul(ps[:], wq_sb[:, di, ci, :], xqo[ci][:],
                                     start=(ci == 0), stop=(ci == CT - 1))
                nc.vector.tensor_scalar_add(
                    out=qt[di][:, q * 512:(q + 1) * 512], in0=ps[:],
                    scalar1=bq_sb[:, di:di + 1])

        for cm in (psA_mm_cm, psA_t_cm, wqkv_cm, quad_cm, wA_cm):
            cm.__exit__(None, None, None)

        # =========== Long-lived pools for B/C/D/E ========================
        maskp_cm = tc.tile_pool(name="maskp", bufs=1)
        maskp = maskp_cm.__enter__()
        avn_cm = tc.tile_pool(name="avnp", bufs=2)
        avnp = avn_cm.__enter__()
        x2_cm = tc.tile_pool(name="x2p", bufs=1)
        x2p = x2_cm.__enter__()
        x2n_cm = tc.tile_pool(name="x2np", bufs=1)
        x2np = x2n_cm.__enter__()
        h_cm = tc.tile_pool(name="hp", bufs=1)
        hpool = h_cm.__enter__()
        wB_cm = tc.tile_pool(name="workB", bufs=2)
        wB = wB_cm.__enter__()
        wR_cm = tc.tile_pool(name="rec", bufs=2)
        wR = wR_cm.__enter__()
        wk2_cm = tc.tile_pool(name="work2", bufs=2)
        wk2 = wk2_cm.__enter__()
        psB_sc_cm = tc.tile_pool(name="psB_sc", bufs=2, space="PSUM")
        psB_sc = psB_sc_cm.__enter__()

        MNS = max(ns)

        def load_masks(Bk):
            n_s, mo = ns[Bk], moffs[Bk]
            mk = maskp.tile([128, MNS, 256], BF16, tag="masks")
            nc.sync.dma_start(
                out=mk[:, 0:n_s, :],
                in_=masks_d[mo:mo + n_s].rearrange("t p f -> p t f"))
            return mk

        def attn_block(Bk, ht, av_pool, mk, avn_t, pend, tails):
            """One (query-block, head-pair) attention block. Emits the
            deferred norm_tail `pend` at i==2 if given."""
            n_s = ns[Bk]
            ps_av = av_pool.tile([128, 2 * W], F32, tag="ps_av",
                                 name="ps_av")

            def av_pair(pi, pex, plo):
                for hp in range(2):
                    nc.tensor.matmul(ps_av[0:65, hp * W + plo:(hp + 1) * W],
                                     v_sb[pi][:, 2 * ht + hp, :],
                                     pex[:, hp * W + plo:(hp + 1) * W],
                                     start=(pi == 0),
                                     stop=(pi == n_s - 1))

            prev = None
            for i in range(n_s):
                lo = _ja(i, Bk) * 128
                ps_sc = psB_sc.tile([128, 2 * W], F32, tag="ps_sc",
                                    name="ps_sc")
                for hp in range(2):
                    nc.tensor.matmul(
                        ps_sc[:, hp * W + lo:(hp + 1) * W],
                        kt[ht][64 * hp:64 * hp + 64,
                               i * 128:(i + 1) * 128],
                        qt[ht][64 * hp:64 * hp + 64,
                               Bk * W + lo:(Bk + 1) * W],
                        start=True, stop=True)
                ex = wB.tile([128, 2 * W], BF16, tag="exp", name="ex")
                nc.scalar.activation(
                    out=ex[:].rearrange("p (h w) -> p h w", h=2)[:, :, lo:W],
                    in_=ps_sc[:].rearrange("p (h w) -> p h w", h=2)[:, :, lo:W],
                    func=AF.Exp, scale=scale)
                mw = min(W - lo, 256)
                for hp in range(2):
                    nc.vector.tensor_mul(
                        ex[:, hp * W + lo:hp * W + lo + mw],
                        ex[:, hp * W + lo:hp * W + lo + mw],
                        mk[:, i, 0:mw])
                if prev is not None:
                    av_pair(*prev)
                prev = (i, ex, lo)
                if i == 2 and pend is not None:
                    tails(pend)
                    pend = None
            av_pair(*prev)
            if pend is not None:
                tails(pend)
            # 1/sumexp: copy the PSUM rows out, DMA-spread the 1024
            # values across 128 partitions, reciprocal on 8 els/lane,
            # DMA back to row-64 layout (cheap exact recip).
            se = wR.tile([128, 2 * W], BF16, tag="se", bufs=1)
            for hp in range(2):
                nc.vector.tensor_copy(
                    out=se[64:65, hp * W:(hp + 1) * W],
                    in_=ps_av[64:65, hp * W:(hp + 1) * W])
            sep = wR.tile([128, 8], BF16, tag="sep", bufs=1)
            nc.sync.dma_start(out=sep[:], in_=se[64:65, :])
            recp = wR.tile([128, 8], BF16, tag="recp", bufs=1)
            nc.vector.reciprocal(out=recp[:], in_=sep[:])
            rec = wR.tile([128, 2 * W], BF16, tag="rec")
            nc.sync.dma_start(out=rec[64:65, :], in_=recp[:])
            return (ht, Bk, ps_av, rec, avn_t)

        def norm_tail(st):
            t_ht, t_Bk, t_av, t_rec, t_avn = st
            for hp in range(2):
                ps_bc = psB_sc.tile([128, 2 * W], F32, tag="ps_sc",
                                    name="bc")
                nc.tensor.matmul(ps_bc[0:64, 0:W], onesmb[64:65, 0:64],
                                 t_rec[64:65, hp * W:(hp + 1) * W],
                                 start=True, stop=True)
                bc_sb = wR.tile([64, W], F32, tag=f"bc_sb{hp}", bufs=1,
                                name=f"bc_sb{hp}")
                nc.vector.tensor_copy(out=bc_sb[:], in_=ps_bc[0:64, 0:W])
                nc.vector.tensor_mul(
                    t_avn[t_ht][64 * hp:64 * hp + 64, :],
                    t_av[0:64, hp * W:(hp + 1) * W], bc_sb[:])

        # =================== B0: attention block Bk=0 ====================
        psB_av0_cm = tc.tile_pool(name="psB_av0", bufs=2, space="PSUM")
        psB_av0 = psB_av0_cm.__enter__()

        mk0 = load_masks(0)
        avn0 = [avnp.tile([128, W], BF16, tag=f"avn{ci}", name=f"avn{ci}")
                for ci in range(CT)]
        # raw residual x^T (column block 0) straight from DRAM
        x2_0 = [x2p.tile([128, W], F32R, tag=f"x2_{ci}", name=f"x2_{ci}")
                for ci in range(CT)]
        for ci in range(CT):
            nc.sync.dma_start(
                out=x2_0[ci][:],
                in_=xoT_d[ci * 128:(ci + 1) * 128, 0:W].bitcast(F32R))

        pending = None
        for ht in range(HT):
            pending = attn_block(0, ht, psB_av0, mk0, avn0, pending,
                                 norm_tail)
        norm_tail(pending)

        psB_av0_cm.__exit__(None, None, None)

        # ============ B1 (Bk=1) interleaved with C0/D0/E0 ================
        psCDE_cm = tc.tile_pool(name="psCDE", bufs=2, space="PSUM")
        psCDE = psCDE_cm.__enter__()
        psB_av1_cm = tc.tile_pool(name="psB_av1", bufs=1, space="PSUM")
        psB_av1 = psB_av1_cm.__enter__()

        def wo_proj(nb, x2_nb, avn_nb, eis):
            for ei in eis:
                wt = wk2.tile([128, CT, 128], BF16, tag="wot")
                nc.sync.dma_start(out=wt[:], in_=wo_d[ei])
                ps = psCDE.tile([128, W], F32, tag="acc", name="ps_o")
                for ci in range(CT):
                    nc.tensor.matmul(ps[:], wt[:, ci, :],
                                     avn_nb[ci][:],
                                     start=(ci == 0), stop=(ci == CT - 1))
                nc.vector.scalar_tensor_tensor(
                    out=x2_nb[ei][:], in0=ps[:],
                    scalar=boeff_sb[:, ei:ei + 1],
                    in1=x2_nb[ei][:], op0=OP.add, op1=OP.add)

        def ln2(nb, x2_nb, x2n_nb):
            ps_mu = psCDE.tile([128, W], F32, tag="acc", name="ps_mu")
            ps_sq = psCDE.tile([128, W], F32, tag="acc", name="ps_sq")
            for ci in range(CT):
                nc.tensor.matmul(ps_mu[0:1, :], ones_c[:], x2_nb[ci][:],
                                 start=(ci == 0), stop=(ci == CT - 1))
                sq = wk2.tile([128, W], F32R, tag="sq", bufs=1)
                nc.scalar.activation(out=sq[:], in_=x2_nb[ci][:],
                                     func=AF.Square)
                nc.tensor.matmul(ps_sq[0:1, :], ones_c[:], sq[:],
                                 start=(ci == 0), stop=(ci == CT - 1))
            mur = wk2.tile([1, W], F32R, tag="mur", bufs=1)
            nc.vector.tensor_scalar_mul(out=mur[:], in0=ps_mu[0:1, :],
                                        scalar1=1.0 / D)
            mu2 = wk2.tile([1, W], F32, tag="rowa", bufs=1)
            nc.vector.tensor_mul(mu2[:], mur[:], mur[:])
            varr = wk2.tile([1, W], F32, tag="varr", bufs=1)
            nc.vector.tensor_scalar_mul(out=varr[:], in0=ps_sq[0:1, :],
                                        scalar1=1.0 / D)
            nc.vector.tensor_sub(varr[:], varr[:], mu2[:])
            stdr = wk2.tile([1, W], F32, tag="rowa", bufs=1)
            nc.scalar.activation(out=stdr[:], in_=varr[:], func=AF.Sqrt,
                                 bias=eps_row[:])
            rstdr = wk2.tile([1, W], F32R, tag="rstdr", bufs=1)
            nc.vector.reciprocal(out=rstdr[:], in_=stdr[:])
            ps_mub = psCDE.tile([128, W], F32, tag="acc", name="ps_mub")
            nc.tensor.matmul(ps_mub[:], ones_r[:], mur[:],
                             start=True, stop=True)
            ps_rsb = psCDE.tile([128, W], F32, tag="acc", name="ps_rsb")
            nc.tensor.matmul(ps_rsb[:], ones_r[:], rstdr[:],
                             start=True, stop=True)
            for ci in range(CT):
                t1 = wk2.tile([128, W], F32, tag="t1", bufs=1)
                nc.vector.tensor_sub(t1[:], x2_nb[ci][:], ps_mub[:])
                nc.vector.tensor_mul(x2n_nb[ci][:], t1[:], ps_rsb[:])

        def mlp_w1(nb, x2n_nb, h_nb, fs):
            for f in fs:
                wtile = wk2.tile([128, CT, 128], BF16, tag="w1t", bufs=3)
                nc.sync.dma_start(out=wtile[:], in_=w1_d[f])
                ps = psCDE.tile([128, W], F32, tag="acc", name="ps_h")
                for ci in range(CT):
                    nc.tensor.matmul(ps[:], wtile[:, ci, :], x2n_nb[ci][:],
                                     start=(ci == 0), stop=(ci == CT - 1))
                nc.scalar.activation(out=h_nb[f][:], in_=ps[:],
                                     func=AF.Relu, bias=b1_sb[:, f:f + 1])

        def mlp_w2(nb, h_nb, x2_nb, eis):
            FH = FT // 4
            for ei in eis:
                ps = psCDE.tile([128, W], F32, tag="acc", name="ps_o2")
                for half in range(4):
                    wtile = wk2.tile([128, FH, 128], BF16, tag="w2t", bufs=3)
                    nc.sync.dma_start(
                        out=wtile[:],
                        in_=w2_d[ei][:, half * FH:(half + 1) * FH, :])
                    for fh in range(FH):
                        f = half * FH + fh
                        nc.tensor.matmul(ps[:], wtile[:, fh, :],
                                         h_nb[f][:],
                                         start=(f == 0), stop=(f == FT - 1))
                osb = wk2.tile([128, W], F32, tag="osb", bufs=1)
                nc.vector.scalar_tensor_tensor(
                    out=osb[:], in0=ps[:], scalar=b2_sb[:, ei:ei + 1],
                    in1=x2_nb[ei][:], op0=OP.add, op1=OP.add)
                nc.sync.dma_start(
                    out=out_d[ei * 128:(ei + 1) * 128, nb * W:(nb + 1) * W],
                    in_=osb[:])

        mk1 = load_masks(1)
        avn1 = [avnp.tile([128, W], BF16, tag=f"avn{ci}", name=f"avn{ci}b")
                for ci in range(CT)]
        x2n_0 = [x2np.tile([128, W], BF16, tag=f"x2n{ci}",
                           name=f"x2n{ci}") for ci in range(CT)]
        h_0 = [hpool.tile([128, W], BF16, tag=f"h{f}", name=f"h{f}")
               for f in range(FT)]

        chunks = [
            lambda: wo_proj(0, x2_0, avn0, range(CT)),
            lambda: ln2(0, x2_0, x2n_0),
            lambda: mlp_w1(0, x2n_0, h_0, range(0, 8)),
            lambda: mlp_w1(0, x2n_0, h_0, range(8, 16)),
            lambda: mlp_w1(0, x2n_0, h_0, range(16, 24)),
            lambda: mlp_w1(0, x2n_0, h_0, range(24, 32)),
            lambda: mlp_w2(0, h_0, x2_0, range(0, 4)),
            lambda: mlp_w2(0, h_0, x2_0, range(4, 8)),
        ]
        for ht in range(HT):
            st = attn_block(1, ht, psB_av1, mk1, avn1, None, norm_tail)
            chunks[ht]()
            norm_tail(st)

        # ===================== tail: C1 / D1 / E1 ========================
        x2_1 = [x2p.tile([128, W], F32R, tag=f"x2_{ci}", name=f"x2_{ci}b")
                for ci in range(CT)]
        for ci in range(CT):
            nc.sync.dma_start(
                out=x2_1[ci][:],
                in_=xoT_d[ci * 128:(ci + 1) * 128, W:2 * W].bitcast(F32R))
        wo_proj(1, x2_1, avn1, range(CT))
        x2n_1 = [x2np.tile([128, W], BF16, tag=f"x2n{ci}",
                           name=f"x2n{ci}b") for ci in range(CT)]
        ln2(1, x2_1, x2n_1)
        h_1 = [hpool.tile([128, W], BF16, tag=f"h{f}", name=f"h{f}b")
               for f in range(FT)]
        mlp_w1(1, x2n_1, h_1, range(FT))
        mlp_w2(1, h_1, x2_1, range(CT))

        for cm in (psB_av1_cm, psCDE_cm, psB_sc_cm, wk2_cm, wR_cm, wB_cm,
                   h_cm, x2n_cm, x2_cm, avn_cm, maskp_cm, attio_cm,
                   consts_cm):
            cm.__exit__(None, None, None)

    nc.compile()
    return nc, g


def make_masks(p, n_own):
    """Per-parity boundary masks [NS_TOT, 128, 512] covering the two
    chunks starting at ja(i, Bk) (bf16 0/1)."""
    NB = n_own // 4
    out = []
    for Bk in range(NB):
        n_s = 8 * Bk + 8
        m = np.ones((n_s, 128, 256), np.float32)
        for i in range(n_s):
            ja = _ja(i, Bk)
            sg = 128 * i + np.arange(128)[:, None]
            for k in range(2):
                j = ja + k
                if j > 3:
                    continue
                gidx = p + 2 * (4 * Bk + j)
                lg = 128 * gidx + np.arange(128)[None, :]
                m[i][:, 128 * k:128 * (k + 1)] = (sg <= lg)
        out.append(m)
    return np.concatenate(out, 0).astype(BF)


def _tile_lhsT(wmat):
    """[K, M] -> [m, p, c, q] with out[m, p, c, q] = wmat[128c+p, 128m+q]."""
    K, M = wmat.shape
    CT, MT = K // 128, M // 128
    w = wmat.reshape(CT, 128, MT, 128)
    return np.ascontiguousarray(w.transpose(2, 1, 0, 3))


def prep_in_maps(inputs, L=L_, D=D_, H=H_, DFF=DFF_, Bn=B_):
    f64 = lambda k: np.asarray(inputs[k], np.float64)
    X = np.asarray(inputs["X"], np.float32)
    WQ, WK, WV, WO = f64("WQ"), f64("WK"), f64("WV"), f64("WO")
    W1, W2 = f64("W1"), f64("W2")
    bQ, bK, bV, bO = f64("bQ"), f64("bK"), f64("bV"), f64("bO")
    b1, b2 = f64("b1"), f64("b2")
    g1, be1, g2, be2 = f64("g1"), f64("be1"), f64("g2"), f64("be2")

    g = _derived(L, D, H, DFF)
    CT, FT, n_own = g["CT"], g["FT"], g["n_own"]

    # fold LayerNorm affine transforms into the downstream weights
    WQf, bQf = g1[:, None] * WQ, bQ + be1 @ WQ
    WKf, bKf = g1[:, None] * WK, bK + be1 @ WK
    WVf, bVf = g1[:, None] * WV, bV + be1 @ WV
    boeff = bO + WO.T @ bVf
    # scale h by 4 (into fp8's normal range); fold 1/4 into W2
    HS = 4.0
    W1f, b1f = g2[:, None] * W1 * HS, (b1 + be2 @ W1) * HS
    W2 = W2 / HS

    c32 = lambda a: np.ascontiguousarray(a).astype(np.float32)
    wq_t = np.ascontiguousarray(
        _tile_lhsT(WQf).transpose(1, 0, 2, 3)).astype(BF)
    wk_t = np.ascontiguousarray(
        _tile_lhsT(WKf).transpose(1, 0, 2, 3)).astype(BF)
    wv_r = np.ascontiguousarray(
        WVf.reshape(CT, 128, D).transpose(1, 0, 2)).astype(BF)
    wo_t = _tile_lhsT(WO).astype(BF)
    w1_t = _tile_lhsT(W1f).astype(BF)
    w2_t = _tile_lhsT(W2).astype(BF)

    def cols(v, nt):
        return c32(np.reshape(v, (nt, 128)).T)

    common = dict(
        ident=np.eye(128, dtype=np.float32),
        wq=wq_t, wk=wk_t, wv=wv_r, wo=wo_t, w1=w1_t, w2=w2_t,
        bqc=cols(bQf, CT), bkc=cols(bKf, CT), b1c=cols(b1f, FT),
        boeffc=cols(boeff, CT), b2c=cols(b2, CT),
        onescv=np.ones((128, 1), np.float32),
        onesrv=np.ones((1, 128), np.float32),
    )
    masks_by_p = [make_masks(p, n_own) for p in range(2)]

    in_maps = []
    for core in range(2 * Bn):
        b, p = core // 2, core % 2
        own_rows = np.concatenate(
            [np.arange(128 * (p + 2 * k), 128 * (p + 2 * k) + 128)
             for k in range(n_own)])
        m = dict(common)
        m["x"] = np.ascontiguousarray(X[b])
        m["x_own"] = np.ascontiguousarray(X[b][own_rows])
        m["x_ownT"] = np.ascontiguousarray(X[b][own_rows].T)
        m["masks"] = masks_by_p[p]
        in_maps.append(m)
    return in_maps


def gather(results, L=L_, D=D_, Bn=B_):
    n_own = (L // 128) // 2
    out = np.empty((Bn, L, D), np.float32)
    for core, r in enumerate(results):
        b, p = core // 2, core % 2
        part = np.ascontiguousarray(r["outT"].T)
        for k in range(n_own):
            out[b, 128 * (p + 2 * k):128 * (p + 2 * k) + 128, :] = \
                part[128 * k:128 * (k + 1), :]
    return out


_NC_CACHE = {}


def get_nc():
    if "nc" not in _NC_CACHE:
        _NC_CACHE["nc"] = build_nc()
    return _NC_CACHE["nc"]


def kernel(**inputs) -> np.ndarray:
    nc, _ = get_nc()
    in_maps = prep_in_maps(inputs)
    res = run_bass_kernel_spmd(nc, in_maps, list(range(N_CORES)))
    return gather(res.results)
